# revision 6
# baseline (speedup 1.0000x reference)
"""Expert-parallel MoE (top-2 of 8 experts) Trainium2 Bass kernel.

Problem: tokens (2,1024,768), 8 experts with SwiGLU-style FFN
(H=3072), token-choice top-2 routing. Only routed (token, expert)
pairs contribute; the host gathers each expert's tokens, the 8 cores
each run one expert's FFN, and the host scatter-adds the combine.

Compute strategy (fp8 DoubleRow, 3-term split):
  Every matmul operand A is represented as A ~ A_hi + A_lo where
  A_hi = f8(A*s), A_lo = f8(A*s - A_hi), f8 = e4m3 round-to-nearest
  (the pair reproduces A*s to ~2^-8 relative). A matmul A@B is then
  computed as  A_hi@B_hi + A_lo@B_hi + A_hi@B_lo  (the dropped
  A_lo@B_lo term is ~2^-16) using fp8e4 DoubleRow matmuls, which the
  PE runs at 0.5 cycles/row -- 2 contraction chunks (K=256) per
  instruction.  Per 128-K-chunk that is 1.5 slot-pairs = 0.75 cycles
  vs 1.0 for fp32r, a 25% PE-time cut at bf16-grade accuracy, and
  fp8 weight pairs halve HBM traffic vs fp32.

  Layout: weights are host-packed as [P, pair, {hi,lo}, slot, P] so
  both DoubleRow slots of one instruction read adjacent K-chunks of
  the same hi/lo plane; x/U hi and lo live in separate [P, K, C]
  tiles sliced [:, 2p:2p+2, cols].

  Scales are powers of two folded into the activation path:
  gelu(G) = act(Gelu, scale=1/(SX*SG)); U*SU = (V_psum * BETA) * t1
  on the DVE; the final 1/(SU*SO) descale rides the host combine.
"""

import numpy as np
import ml_dtypes

import concourse.bass as bass
import concourse.mybir as mybir
import concourse.tile as tile
from concourse.bacc import Bacc
from concourse.bass import ds
from concourse.bass_utils import run_bass_kernel_spmd

# Problem constants (fixed by the grading harness's input shapes).
B, N, D, E, H = 2, 1024, 768, 8, 3072
T = B * N
P = 128
KD = D // P      # 6 chunks of the model dim
KH = H // P      # 24 chunks of the hidden dim
KDP = KD // 2    # 3 chunk-pairs (DoubleRow contracts 2 chunks/instr)
KHP = KH // 2    # 12 chunk-pairs
N_CORES = 8

F8 = ml_dtypes.float8_e4m3

# Power-of-2 quantization scales: put |max| of each tensor near ~100
# (e4m3 max finite 224). max|x|~5.1, max|Wg|=max|Wv|~0.0395,
# max|Wo|~7.9e-4, max|U|~8.6 on the reference distribution.
SX = 16.0
SG = 2048.0
SV = 2048.0
SO = 131072.0
SU = 8.0
CG = 1.0 / (SX * SG)      # PSUM -> true G, applied inside the Gelu
BETA = SU / (SX * SV)     # PSUM V -> U*SU when multiplied with gelu(G)
DESCALE = 1.0 / (SU * SO)  # applied on host during combine

_NC_CACHE: dict = {}
LAST_RESULTS = None  # BassKernelResults of the most recent kernel() call


def _build_nc(
    C: int,
    NQ: int,
    act: str = "Gelu",
    warmup: int = 24,
    ps1_bufs: int = 6,
    ps2_bufs: int = 2,
    w1_bufs: int = 3,
    w2_bufs: int = 7,
    tmp_bufs: int = 6,
    wo_pre: int = 4,
) -> bass.Bass:
    """One-expert fp8 FFN over C (padded) tokens; SPMD across 8 cores."""
    assert C % NQ == 0
    CQ = C // NQ
    assert CQ <= 256, "DoubleRow moving free dim is 2*CQ <= 512"
    assert CQ % 4 == 0
    f32 = mybir.dt.float32
    f8 = mybir.dt.float8e4
    GELU = getattr(mybir.ActivationFunctionType, act)
    COPY = mybir.ActivationFunctionType.Copy
    DR = mybir.MatmulPerfMode.DoubleRow
    MUL = mybir.AluOpType.mult
    SUB = mybir.AluOpType.subtract

    nc = Bacc()
    # xhi/xlo[d, kd, c] = f8(x_pad[c, kd*128+d] * SX) hi/lo pair
    # wg/wv[mh][d, p, hl, s, h] = f8pair(Wg[mh*128+h, (2p+s)*128+d] * SG)
    # wo[md][h, p, hl, s, d]    = f8pair(Wo[md*128+d, (2p+s)*128+h] * SO)
    # out[md, d, c] = expert_out^T[md*128+d, c] * SU*SO
    xhi_d = nc.declare_dram_parameter("xhi", [P, KD, C], f8, isOutput=False)
    xlo_d = nc.declare_dram_parameter("xlo", [P, KD, C], f8, isOutput=False)
    wg_d = nc.declare_dram_parameter(
        "wg", [KH, P, KDP, 2, 2, P], f8, isOutput=False
    )
    wv_d = nc.declare_dram_parameter(
        "wv", [KH, P, KDP, 2, 2, P], f8, isOutput=False
    )
    wo_d = nc.declare_dram_parameter(
        "wo", [KD, P, KHP, 2, 2, P], f8, isOutput=False
    )
    wrm_d = (
        nc.declare_dram_parameter("wrm", [P, 2, P], f8, isOutput=False)
        if warmup
        else None
    )
    out_d = nc.declare_dram_parameter("out", [KD, P, C], f32, isOutput=True)

    with tile.TileContext(nc) as tc:
        with (
            tc.tile_pool(name="singles", bufs=1) as singles,
            tc.tile_pool(name="w1", bufs=w1_bufs) as w1pool,
            tc.tile_pool(name="w2", bufs=w2_bufs) as w2pool,
            tc.tile_pool(name="tmp", bufs=tmp_bufs) as tmppool,
            tc.tile_pool(name="outp", bufs=3) as outpool,
            tc.tile_pool(name="ps1", bufs=ps1_bufs, space="PSUM") as ps1,
            tc.tile_pool(name="ps2", bufs=ps2_bufs, space="PSUM") as ps2,
        ):
            if warmup:
                # PE clock (HAM) warm-up during the initial DMA wait.
                wz = singles.tile([P, 2, P], f8)
                nc.sync.dma_start(out=wz[:], in_=wrm_d[:])
                pw = ps2.tile([P, P], f32, name="warm", tag="o_ps")
                for _ in range(warmup):
                    nc.tensor.matmul(
                        pw[:], wz[:], wz[:], start=True, stop=True,
                        perf_mode=DR,
                    )

            xhi = singles.tile([P, KD, C], f8)
            nc.sync.dma_start(out=xhi[:], in_=xhi_d[:])
            xlo = singles.tile([P, KD, C], f8)
            nc.sync.dma_start(out=xlo[:], in_=xlo_d[:])
            uhi = singles.tile([P, KH, C], f8)
            ulo = singles.tile([P, KH, C], f8)

            # Phase 1: U = gelu(G)*V per H-chunk; store U as fp8 hi/lo.
            # Phase-2 Wo tiles are prefetched during phase 1 (one every
            # `wo_pre` H-chunks) so the phase transition never waits on DMA.
            wo_tiles = []
            for mh in range(KH):
                wg_t = w1pool.tile([P, KDP, 2, 2, P], f8)
                nc.sync.dma_start(out=wg_t[:], in_=wg_d[mh])
                wv_t = w1pool.tile([P, KDP, 2, 2, P], f8)
                nc.sync.dma_start(out=wv_t[:], in_=wv_d[mh])
                if mh % wo_pre == 0 and len(wo_tiles) < KD:
                    wo_t = w2pool.tile([P, KHP, 2, 2, P], f8)
                    nc.sync.dma_start(out=wo_t[:], in_=wo_d[len(wo_tiles)])
                    wo_tiles.append(wo_t)
                for q in range(NQ):
                    sl = ds(q * CQ, CQ)
                    g_ps = ps1.tile([P, CQ], f32, name="g_ps", tag="gv")
                    v_ps = ps1.tile([P, CQ], f32, name="v_ps", tag="gv")
                    for ps_t, w_t in ((g_ps, wg_t), (v_ps, wv_t)):
                        for p in range(KDP):
                            nc.tensor.matmul(
                                ps_t[:], w_t[:, p, 0],
                                xhi[:, ds(2 * p, 2), sl],
                                start=(p == 0), stop=False, perf_mode=DR,
                            )
                        for p in range(KDP):
                            nc.tensor.matmul(
                                ps_t[:], w_t[:, p, 1],
                                xhi[:, ds(2 * p, 2), sl],
                                start=False, stop=False, perf_mode=DR,
                            )
                        for p in range(KDP):
                            nc.tensor.matmul(
                                ps_t[:], w_t[:, p, 0],
                                xlo[:, ds(2 * p, 2), sl],
                                start=False, stop=(p == KDP - 1),
                                perf_mode=DR,
                            )
                    t1 = tmppool.tile([P, CQ], f32)
                    nc.scalar.activation(
                        out=t1[:], in_=g_ps[:], func=GELU, scale=CG
                    )
                    u32 = tmppool.tile([P, CQ], f32)
                    nc.vector.scalar_tensor_tensor(
                        out=u32[:], in0=v_ps[:], scalar=BETA, in1=t1[:],
                        op0=MUL, op1=MUL,
                    )
                    nc.scalar.activation(
                        out=uhi[:, mh, sl], in_=u32[:], func=COPY
                    )
                    nc.vector.scalar_tensor_tensor(
                        out=ulo[:, mh, sl], in0=u32[:], scalar=1.0,
                        in1=uhi[:, mh, sl], op0=MUL, op1=SUB,
                    )

            # Phase 2: out^T[md] = sum_kh Wo-pairs @ U-pairs
            for md in range(KD):
                wo_t = wo_tiles[md]
                o_t = outpool.tile([P, C], f32)
                for q in range(NQ):
                    sl = ds(q * CQ, CQ)
                    o_ps = ps2.tile([P, CQ], f32, name="o_ps", tag="o_ps")
                    for p in range(KHP):
                        nc.tensor.matmul(
                            o_ps[:], wo_t[:, p, 0],
                            uhi[:, ds(2 * p, 2), sl],
                            start=(p == 0), stop=False, perf_mode=DR,
                        )
                    for p in range(KHP):
                        nc.tensor.matmul(
                            o_ps[:], wo_t[:, p, 1],
                            uhi[:, ds(2 * p, 2), sl],
                            start=False, stop=False, perf_mode=DR,
                        )
                    for p in range(KHP):
                        nc.tensor.matmul(
                            o_ps[:], wo_t[:, p, 0],
                            ulo[:, ds(2 * p, 2), sl],
                            start=False, stop=(p == KHP - 1), perf_mode=DR,
                        )
                    nc.vector.tensor_copy(o_t[:, sl], o_ps[:])
                nc.sync.dma_start(out=out_d[md], in_=o_t[:])

    nc.finalize()
    return nc


def _get_nc(C: int, NQ: int, act: str = "Gelu") -> bass.Bass:
    if (C, NQ, act) not in _NC_CACHE:
        _NC_CACHE[(C, NQ, act)] = _build_nc(C, NQ, act)
    return _NC_CACHE[(C, NQ, act)]


def _capacity(max_cnt: int) -> tuple[int, int]:
    """Pick (C, NQ): C = NQ*CQ >= max_cnt with CQ <= 256 (DoubleRow
    moving-free limit), CQ a multiple of 8."""
    NQ = max(2, -(-max_cnt // 256))
    CQ = -(-(-(-max_cnt // NQ)) // 8) * 8
    return NQ * CQ, NQ


def _f8_pair(a: np.ndarray) -> tuple[np.ndarray, np.ndarray]:
    hi = a.astype(F8)
    lo = (a - hi.astype(np.float32)).astype(F8)
    return hi, lo


def _pack_w(q: np.ndarray, KO: int, KPI: int) -> np.ndarray:
    """q [out=KO*P, in=KPI*2*P] (pre-scaled f32) -> f8 pair layout
    [KO, P(in_part), KPI, {hi,lo}, slot, P(out_col)]."""
    hi, lo = _f8_pair(q)
    # [ko, h, p, s, d] with out=ko*P+h, in=(2p+s)*P+d
    hi_r = hi.reshape(KO, P, KPI, 2, P)
    lo_r = lo.reshape(KO, P, KPI, 2, P)
    st = np.stack([hi_r, lo_r], axis=0)  # [hl, ko, h, p, s, d]
    return np.ascontiguousarray(st.transpose(1, 5, 3, 0, 4, 2))


def _prep_in_maps(x, Wg, Wv, Wo, C, idxs, cnts):
    in_maps = []
    xs = x * SX
    for e in range(E):
        xg = np.zeros((C, D), np.float32)
        xg[: cnts[e]] = xs[idxs[e]]
        xhi, xlo = _f8_pair(xg)
        xhi_h = np.ascontiguousarray(
            xhi.T.reshape(KD, P, C).transpose(1, 0, 2)
        )
        xlo_h = np.ascontiguousarray(
            xlo.T.reshape(KD, P, C).transpose(1, 0, 2)
        )
        in_maps.append(
            {
                "xhi": xhi_h,
                "xlo": xlo_h,
                "wg": _pack_w(Wg[e] * SG, KH, KDP),
                "wv": _pack_w(Wv[e] * SV, KH, KDP),
                "wo": _pack_w(Wo[e] * SO, KD, KHP),
                "wrm": np.zeros((P, 2, P), F8),
            }
        )
    return in_maps


def kernel(
    tokens, dispatch_weights, combine_weights, Wg, Wv, Wo, scale, **run_kwargs
):
    x = np.ascontiguousarray(np.asarray(tokens, np.float32).reshape(T, D))
    dw = np.asarray(dispatch_weights, np.float32).reshape(T, E)
    cw = np.asarray(combine_weights, np.float32).reshape(T, E)
    Wg = np.ascontiguousarray(np.asarray(Wg, np.float32))
    Wv = np.ascontiguousarray(np.asarray(Wv, np.float32))
    Wo = np.ascontiguousarray(np.asarray(Wo, np.float32))
    scale = np.asarray(scale, np.float32)

    mask = dw > 0
    comb = np.where(mask, cw, 0.0).astype(np.float32)
    idxs = [np.nonzero(mask[:, e])[0] for e in range(E)]
    cnts = [len(i) for i in idxs]
    C, NQ = _capacity(max(cnts))

    nc = _get_nc(C, NQ)
    in_maps = _prep_in_maps(x, Wg, Wv, Wo, C, idxs, cnts)
    res = run_bass_kernel_spmd(
        nc, in_maps, core_ids=list(range(N_CORES)), **run_kwargs
    )
    global LAST_RESULTS
    LAST_RESULTS = res

    y = np.zeros((T, D), np.float32)
    for e in range(E):
        outT = np.asarray(res.results[e]["out"]).reshape(D, C)
        w = (comb[idxs[e], e] * scale[e] * DESCALE).astype(np.float32)
        y[idxs[e]] += outT.T[: cnts[e]] * w[:, None]
    return y.reshape(B, N, D)


# revision 22
# speedup vs baseline: 1.0349x; 1.0349x over previous
"""Expert-parallel MoE (top-2 of 8 experts) Trainium2 Bass kernel.

Problem: tokens (2,1024,768), 8 experts with SwiGLU-style FFN
(H=3072), token-choice top-2 routing. Only routed (token, expert)
pairs contribute; the host gathers each expert's tokens, the 8 cores
each run one expert's FFN, and the host scatter-adds the combine.

Compute strategy (fp8 DoubleRow, 3-term split):
  Every matmul operand A is represented as A ~ A_hi + A_lo where
  A_hi = f8(A*s), A_lo = f8(A*s - A_hi), f8 = e4m3 round-to-nearest
  (the pair reproduces A*s to ~2^-8 relative). A matmul A@B is then
  computed as  A_hi@B_hi + A_lo@B_hi + A_hi@B_lo  (the dropped
  A_lo@B_lo term is ~2^-16) using fp8e4 DoubleRow matmuls, which the
  PE runs at 0.5 cycles/row -- 2 contraction chunks (K=256) per
  instruction.  Per 128-K-chunk that is 1.5 slot-pairs = 0.75 cycles
  vs 1.0 for fp32r, a 25% PE-time cut at bf16-grade accuracy, and
  fp8 weight pairs halve HBM traffic vs fp32.

  Layout: weights are host-packed as [P, pair, {hi,lo}, slot, P] so
  both DoubleRow slots of one instruction read adjacent K-chunks of
  the same hi/lo plane; x/U hi and lo live in separate [P, K, C]
  tiles sliced [:, 2p:2p+2, cols].

  Scales are powers of two folded into the activation path:
  gelu(G) = act(Gelu, scale=1/(SX*SG)); U*SU = (V_psum * BETA) * t1
  on the DVE; the final 1/(SU*SO) descale rides the host combine.
"""

import numpy as np
import ml_dtypes

import concourse.bass as bass
import concourse.mybir as mybir
import concourse.tile as tile
from concourse.bacc import Bacc
from concourse.bass import ds
from concourse.bass_utils import run_bass_kernel_spmd

# Problem constants (fixed by the grading harness's input shapes).
B, N, D, E, H = 2, 1024, 768, 8, 3072
T = B * N
P = 128
KD = D // P      # 6 chunks of the model dim
KH = H // P      # 24 chunks of the hidden dim
KDP = KD // 2    # 3 chunk-pairs (DoubleRow contracts 2 chunks/instr)
KHP = KH // 2    # 12 chunk-pairs
N_CORES = 8

F8 = ml_dtypes.float8_e4m3

# Power-of-2 quantization scales: put |max| of each tensor near ~100
# (e4m3 max finite 224). max|x|~5.1, max|Wg|=max|Wv|~0.0395,
# max|Wo|~7.9e-4, max|U|~8.6 on the reference distribution.
SX = 16.0
SG = 2048.0
SV = 2048.0
SO = 131072.0
SU = 8.0
CG = 1.0 / (SX * SG)      # PSUM -> true G, applied inside the Gelu
BETA = SU / (SX * SV)     # PSUM V -> U*SU when multiplied with gelu(G)
DESCALE = 1.0 / (SU * SO)  # applied on host during combine

_NC_CACHE: dict = {}
LAST_RESULTS = None  # BassKernelResults of the most recent kernel() call


def _build_nc(
    C: int,
    NQ: int,
    act: str = "Gelu",
    warmup: int = 30,
    ps1_bufs: int = 5,
    ps2_bufs: int = 3,
    w1_bufs: int = 3,
    w2_bufs: int = 6,
    tmp_bufs: int = 6,
    wo_pre: int = 4,
    psum_out_dma: bool = True,
) -> bass.Bass:
    """One-expert fp8 FFN over C (padded) tokens; SPMD across 8 cores."""
    assert C % NQ == 0
    CQ = C // NQ
    assert CQ <= 256, "DoubleRow moving free dim is 2*CQ <= 512"
    assert CQ % 4 == 0
    f32 = mybir.dt.float32
    f8 = mybir.dt.float8e4
    GELU = getattr(mybir.ActivationFunctionType, act)
    COPY = mybir.ActivationFunctionType.Copy
    DR = mybir.MatmulPerfMode.DoubleRow
    MUL = mybir.AluOpType.mult
    SUB = mybir.AluOpType.subtract

    nc = Bacc()
    # x2[q][d, hl, kd, cq] = f8pair(x_pad[q*CQ+cq, kd*128+d] * SX)
    #   (q-major so the q=0 block is one small contiguous head DMA)
    # wg/wv[mh][d, p, hl, s, h] = f8pair(Wg[mh*128+h, (2p+s)*128+d] * SG)
    # wo[md][h, p, hl, s, d]    = f8pair(Wo[md*128+d, (2p+s)*128+h] * SO)
    # out[md, d, c] = expert_out^T[md*128+d, c] * SU*SO
    x2_d = nc.declare_dram_parameter(
        "x2", [NQ, P, 2, KD, CQ], f8, isOutput=False
    )
    wg_d = nc.declare_dram_parameter(
        "wg", [KH, P, KDP, 2, 2, P], f8, isOutput=False
    )
    wv_d = nc.declare_dram_parameter(
        "wv", [KH, P, KDP, 2, 2, P], f8, isOutput=False
    )
    wo_d = nc.declare_dram_parameter(
        "wo", [KD, P, KHP, 2, 2, P], f8, isOutput=False
    )
    out_d = nc.declare_dram_parameter("out", [KD, P, C], f32, isOutput=True)

    with tile.TileContext(nc) as tc:
        with (
            tc.tile_pool(name="singles", bufs=1) as singles,
            tc.tile_pool(name="w1", bufs=w1_bufs) as w1pool,
            tc.tile_pool(name="w2", bufs=w2_bufs) as w2pool,
            tc.tile_pool(name="tmp", bufs=tmp_bufs) as tmppool,
            tc.tile_pool(name="outp", bufs=3) as outpool,
            tc.tile_pool(name="ps1", bufs=ps1_bufs, space="PSUM") as ps1,
            tc.tile_pool(name="ps2", bufs=ps2_bufs, space="PSUM") as ps2,
        ):
            if warmup:
                # PE clock (HAM) warm-up during the initial DMA wait.
                # Weights come from a memset tile (no DMA on the warmup
                # critical path) and the matmuls are CQ-sized so ~56 of
                # them span the ~6us until the first real operands land,
                # carrying the PE p-state ramp.
                wzw = singles.tile([P, 2, P], f8)
                nc.gpsimd.memset(wzw[:], 0)
                wzm = singles.tile([P, 2, 256], f8)
                nc.gpsimd.memset(wzm[:], 0)
                pw = ps2.tile([P, 256], f32, name="warm", tag="o_ps")
                for _ in range(warmup):
                    nc.tensor.matmul(
                        pw[:], wzw[:], wzm[:], start=True, stop=True,
                        perf_mode=DR,
                    )

            x2 = singles.tile([P, NQ, 2, KD, CQ], f8)
            nc.sync.dma_start(out=x2[:, 0], in_=x2_d[0])
            wg_t0 = w1pool.tile([P, KDP, 2, 2, P], f8)
            nc.sync.dma_start(out=wg_t0[:], in_=wg_d[0])
            wv_t0 = w1pool.tile([P, KDP, 2, 2, P], f8)
            nc.sync.dma_start(out=wv_t0[:], in_=wv_d[0])
            for q in range(1, NQ):
                nc.sync.dma_start(out=x2[:, q], in_=x2_d[q])
            uhi = singles.tile([P, KH, C], f8)
            ulo = singles.tile([P, KH, C], f8)

            # Phase 1: U = gelu(G)*V per H-chunk; store U as fp8 hi/lo.
            # Phase-2 Wo tiles are prefetched during phase 1 (one every
            # `wo_pre` H-chunks) so the phase transition never waits on DMA.
            wo_tiles = []
            for mh in range(KH):
                if mh == 0:
                    wg_t, wv_t = wg_t0, wv_t0
                else:
                    wg_t = w1pool.tile([P, KDP, 2, 2, P], f8)
                    nc.sync.dma_start(out=wg_t[:], in_=wg_d[mh])
                    wv_t = w1pool.tile([P, KDP, 2, 2, P], f8)
                    nc.sync.dma_start(out=wv_t[:], in_=wv_d[mh])
                    if mh % wo_pre == 0 and len(wo_tiles) < KD:
                        wo_t = w2pool.tile([P, KHP, 2, 2, P], f8)
                        nc.sync.dma_start(out=wo_t[:], in_=wo_d[len(wo_tiles)])
                        wo_tiles.append(wo_t)
                for q in range(NQ):
                    sl = ds(q * CQ, CQ)
                    g_ps = ps1.tile([P, CQ], f32, name="g_ps", tag="gv")
                    v_ps = ps1.tile([P, CQ], f32, name="v_ps", tag="gv")
                    for ps_t, w_t in ((g_ps, wg_t), (v_ps, wv_t)):
                        for p in range(KDP):
                            nc.tensor.matmul(
                                ps_t[:], w_t[:, p, 0],
                                x2[:, q, 0, ds(2 * p, 2)],
                                start=(p == 0), stop=False, perf_mode=DR,
                            )
                        for p in range(KDP):
                            nc.tensor.matmul(
                                ps_t[:], w_t[:, p, 1],
                                x2[:, q, 0, ds(2 * p, 2)],
                                start=False, stop=False, perf_mode=DR,
                            )
                        for p in range(KDP):
                            nc.tensor.matmul(
                                ps_t[:], w_t[:, p, 0],
                                x2[:, q, 1, ds(2 * p, 2)],
                                start=False, stop=(p == KDP - 1),
                                perf_mode=DR,
                            )
                    t1 = tmppool.tile([P, CQ], f32)
                    nc.scalar.activation(
                        out=t1[:], in_=g_ps[:], func=GELU, scale=CG
                    )
                    u32 = tmppool.tile([P, CQ], f32)
                    nc.vector.scalar_tensor_tensor(
                        out=u32[:], in0=v_ps[:], scalar=BETA, in1=t1[:],
                        op0=MUL, op1=MUL,
                    )
                    nc.scalar.activation(
                        out=uhi[:, mh, sl], in_=u32[:], func=COPY
                    )
                    nc.vector.scalar_tensor_tensor(
                        out=ulo[:, mh, sl], in0=u32[:], scalar=1.0,
                        in1=uhi[:, mh, sl], op0=MUL, op1=SUB,
                    )

            # Phase 2: out^T[md] = sum_kh Wo-pairs @ U-pairs
            for md in range(KD):
                if md < len(wo_tiles):
                    wo_t = wo_tiles[md]
                else:
                    wo_t = w2pool.tile([P, KHP, 2, 2, P], f8)
                    nc.sync.dma_start(out=wo_t[:], in_=wo_d[md])
                o_t = outpool.tile([P, C], f32)
                for q in range(NQ):
                    sl = ds(q * CQ, CQ)
                    o_ps = ps2.tile([P, CQ], f32, name="o_ps", tag="o_ps")
                    for p in range(KHP):
                        nc.tensor.matmul(
                            o_ps[:], wo_t[:, p, 0],
                            uhi[:, ds(2 * p, 2), sl],
                            start=(p == 0), stop=False, perf_mode=DR,
                        )
                    for p in range(KHP):
                        nc.tensor.matmul(
                            o_ps[:], wo_t[:, p, 1],
                            uhi[:, ds(2 * p, 2), sl],
                            start=False, stop=False, perf_mode=DR,
                        )
                    for p in range(KHP):
                        nc.tensor.matmul(
                            o_ps[:], wo_t[:, p, 0],
                            ulo[:, ds(2 * p, 2), sl],
                            start=False, stop=(p == KHP - 1), perf_mode=DR,
                        )
                    nc.vector.tensor_copy(o_t[:, sl], o_ps[:])
                    if psum_out_dma:
                        nc.sync.dma_start(out=out_d[md, :, sl], in_=o_t[:, sl])
                if not psum_out_dma:
                    nc.sync.dma_start(out=out_d[md], in_=o_t[:])

    nc.finalize()
    return nc


def _get_nc(C: int, NQ: int, act: str = "Gelu") -> bass.Bass:
    if (C, NQ, act) not in _NC_CACHE:
        _NC_CACHE[(C, NQ, act)] = _build_nc(C, NQ, act)
    return _NC_CACHE[(C, NQ, act)]


def _capacity(max_cnt: int) -> tuple[int, int]:
    """Pick (C, NQ): C = NQ*CQ >= max_cnt with CQ <= 256 (DoubleRow
    moving-free limit), CQ a multiple of 8."""
    NQ = max(2, -(-max_cnt // 256))
    CQ = -(-(-(-max_cnt // NQ)) // 8) * 8
    return NQ * CQ, NQ


def _f8_pair(a: np.ndarray) -> tuple[np.ndarray, np.ndarray]:
    hi = a.astype(F8)
    lo = (a - hi.astype(np.float32)).astype(F8)
    return hi, lo


def _pack_w(q: np.ndarray, KO: int, KPI: int) -> np.ndarray:
    """q [out=KO*P, in=KPI*2*P] (pre-scaled f32) -> f8 pair layout
    [KO, P(in_part), KPI, {hi,lo}, slot, P(out_col)]."""
    hi, lo = _f8_pair(q)
    # [ko, h, p, s, d] with out=ko*P+h, in=(2p+s)*P+d
    hi_r = hi.reshape(KO, P, KPI, 2, P)
    lo_r = lo.reshape(KO, P, KPI, 2, P)
    st = np.stack([hi_r, lo_r], axis=0)  # [hl, ko, h, p, s, d]
    return np.ascontiguousarray(st.transpose(1, 5, 3, 0, 4, 2))


def _prep_in_maps(x, Wg, Wv, Wo, C, idxs, cnts, NQ=None):
    if NQ is None:
        NQ = _capacity(max(cnts))[1]
    CQ = C // NQ
    in_maps = []
    xs = x * SX
    for e in range(E):
        xg = np.zeros((C, D), np.float32)
        xg[: cnts[e]] = xs[idxs[e]]
        xhi, xlo = _f8_pair(xg)
        # [hl, kd, d, q, cq] -> [q, d, hl, kd, cq]
        st = np.stack(
            [xhi.T.reshape(KD, P, NQ, CQ), xlo.T.reshape(KD, P, NQ, CQ)],
            axis=0,
        )
        x2 = np.ascontiguousarray(st.transpose(3, 2, 0, 1, 4))
        in_maps.append(
            {
                "x2": x2,
                "wg": _pack_w(Wg[e] * SG, KH, KDP),
                "wv": _pack_w(Wv[e] * SV, KH, KDP),
                "wo": _pack_w(Wo[e] * SO, KD, KHP),
            }
        )
    return in_maps


def kernel(
    tokens, dispatch_weights, combine_weights, Wg, Wv, Wo, scale, **run_kwargs
):
    x = np.ascontiguousarray(np.asarray(tokens, np.float32).reshape(T, D))
    dw = np.asarray(dispatch_weights, np.float32).reshape(T, E)
    cw = np.asarray(combine_weights, np.float32).reshape(T, E)
    Wg = np.ascontiguousarray(np.asarray(Wg, np.float32))
    Wv = np.ascontiguousarray(np.asarray(Wv, np.float32))
    Wo = np.ascontiguousarray(np.asarray(Wo, np.float32))
    scale = np.asarray(scale, np.float32)

    mask = dw > 0
    comb = np.where(mask, cw, 0.0).astype(np.float32)
    idxs = [np.nonzero(mask[:, e])[0] for e in range(E)]
    cnts = [len(i) for i in idxs]
    C, NQ = _capacity(max(cnts))

    nc = _get_nc(C, NQ)
    in_maps = _prep_in_maps(x, Wg, Wv, Wo, C, idxs, cnts)
    res = run_bass_kernel_spmd(
        nc, in_maps, core_ids=list(range(N_CORES)), **run_kwargs
    )
    global LAST_RESULTS
    LAST_RESULTS = res

    y = np.zeros((T, D), np.float32)
    for e in range(E):
        outT = np.asarray(res.results[e]["out"]).reshape(D, C)
        w = (comb[idxs[e], e] * scale[e] * DESCALE).astype(np.float32)
        y[idxs[e]] += outT.T[: cnts[e]] * w[:, None]
    return y.reshape(B, N, D)
